# revision 44
# baseline (speedup 1.0000x reference)
"""BertAttention Trainium2 kernel — 8-core SPMD, v2.

Sharding: each core owns 2 heads (128 of 1024 feature dims) for QKV +
attention, and a strided 512-token slice (256 tokens from each batch)
for the output projection + LayerNorm.

Key optimizations over v1:
  - score matmuls for the 2 local heads run CONCURRENTLY in the PE array
    (row-tiled at partitions 0-63 / 64-127), 2x score throughput
  - softmax exp (the ACT-engine bottleneck) is pipelined stall-free via a
    3-deep PSUM ring; ctx matmuls lag one i-chunk behind the score/exp
    stream; projections and out-proj run as PE filler under the exp stream
  - ctx keeps a ones-column in V so softmax row-sums are free
  - AllToAll (not AllGather) exchanges exactly the needed ctx slices;
    one per batch, the first fully overlapped with batch-1 attention
  - LayerNorm rsqrt via Newton iterations on DVE (no ACT table switch
    mid-exp-stream); out-proj + LN of batch-0 tokens overlap batch-1
  - last i-chunk of each batch interleaves ctx into the exp stream so the
    collective can start right after the last exp
"""

import os
import sys

for _p in ("/opt/trn_rl_repo", "/root/.axon_site/_ro/trn_rl_repo"):
    if os.path.isdir(_p) and _p not in sys.path:
        sys.path.append(_p)

import ml_dtypes
import numpy as np

try:
    import antenv.axon_hooks  # noqa: F401
except Exception:
    import types as _types
    try:
        import antenv as _antenv
        _m = _types.ModuleType("antenv.axon_hooks")
        _m._hook = None
        _m.set_axon_ntff_profile_hook = lambda h, _m=_m: setattr(_m, "_hook", h)
        _m.get_axon_ntff_profile_hook = lambda _m=_m: _m._hook
        sys.modules["antenv.axon_hooks"] = _m
        _antenv.axon_hooks = _m
    except Exception:
        pass

import concourse.bass as bass
import concourse.tile as tile
from concourse import bacc, mybir
from concourse.bass_utils import run_bass_kernel_spmd

F32 = mybir.dt.float32
I32 = mybir.dt.int32
BF16 = mybir.dt.bfloat16
BF16_NP = ml_dtypes.bfloat16

NCORES = 8
H = 16   # heads total
DH = 64  # head dim
LN_EPS = 1e-12
MAGIC = 0x5F3759DF


def build_bert_kernel(S=2048, B=2, D=1024, debug_dumps=False):
    P = 128
    NTOK = S * B             # batch-major tokens
    TPB = S // NCORES        # tokens per core per batch (256)
    TPC = B * TPB            # tokens per core total (512)
    CCH = D // P             # contraction chunks (8)
    HPC = H // NCORES        # heads per core (2)
    DL = HPC * DH            # local feature dims (128)
    NI = S // 512            # i-chunks per batch (4)
    NG = 8                   # exp groups per i-chunk (2 j-chunks each)
    NJ = S // P              # key chunks per batch (16)
    NT = TPC // P            # out-proj token tiles per core (4)

    nc = bacc.Bacc("TRN2", target_bir_lowering=False, debug=False,
                   num_devices=NCORES)

    def din(name, shape, dt=F32):
        return nc.dram_tensor(name, list(shape), dt, kind="ExternalInput").ap()

    xqT = din("xqT", (D, NTOK), BF16)
    xkT = din("xkT", (D, NTOK), BF16)
    xvT = din("xvT", (D, NTOK), BF16)
    wqT = din("wqT", (D, DL), BF16)
    wkT = din("wkT", (D, DL), BF16)
    wvT = din("wvT", (D, DL), BF16)
    woT = din("woT", (D, D), BF16)
    bq = din("bq", (DL, 1))
    bk = din("bk", (DL, 1))
    bv = din("bv", (1, DL))
    lnw = din("lnw", (1, D))
    lnb = din("lnb", (1, D))
    resid = din("resid", (TPC, D), BF16)  # xq rows + bo pre-added on host
    out = nc.dram_tensor("out", [TPC, D], F32, kind="ExternalOutput").ap()

    # AllToAll buffers: [dest core, local dims (2 heads), dest's tokens]
    a2a_in = [nc.dram_tensor(f"a2a_in{b}", [NCORES, DL, TPB], BF16).ap()
              for b in range(B)]
    a2a_out = [nc.dram_tensor(f"a2a_out{b}", [NCORES, DL, TPB], BF16).ap()
               for b in range(B)]

    with tile.TileContext(nc) as tc:
        with (
            tc.tile_pool(name="persist", bufs=1) as persist,
            tc.tile_pool(name="small", bufs=1) as small,
            tc.tile_pool(name="xp", bufs=1) as xp,
            tc.tile_pool(name="ep", bufs=1) as ep,
            tc.tile_pool(name="sc_pool", bufs=1, space="PSUM") as sc_pool,
            tc.tile_pool(name="aux_pool", bufs=1, space="PSUM") as aux_pool,
            tc.tile_pool(name="work", bufs=1) as work,
        ):
            # ====== weights + consts (scalar/gpsimd; sync is for x) ======
            wq_sb = persist.tile([P, CCH, DL], BF16)
            wk_sb = persist.tile([P, CCH, DL], BF16)
            wv_sb = persist.tile([P, CCH, DL], BF16)
            for c in range(CCH):
                nc.scalar.dma_start(out=wk_sb[:, c, :],
                                    in_=wkT[c * P:(c + 1) * P, :])
            for c in range(CCH):
                nc.scalar.dma_start(out=wq_sb[:, c, :],
                                    in_=wqT[c * P:(c + 1) * P, :])
            for c in range(CCH):
                nc.gpsimd.dma_start(out=wv_sb[:, c, :],
                                    in_=wvT[c * P:(c + 1) * P, :])
            woT_sb = persist.tile([P, CCH, D], BF16)  # loaded mid-attention

            prime = small.tile([1, 8], F32)
            nc.vector.memset(prime, 0.0)
            nc.scalar.activation(prime, prime,
                                 mybir.ActivationFunctionType.Exp)
            bq_sb = small.tile([DL, 1], F32)
            bk_sb = small.tile([DL, 1], F32)
            nc.scalar.dma_start(out=bq_sb, in_=bq)
            nc.scalar.dma_start(out=bk_sb, in_=bk)
            bv_bc = small.tile([P, DL], F32)
            nc.scalar.dma_start(out=bv_bc, in_=bv.to_broadcast((P, DL)))
            lnw_bc = small.tile([P, D], F32)
            nc.scalar.dma_start(out=lnw_bc, in_=lnw.to_broadcast((P, D)))
            lnb_bc = small.tile([P, D], F32)
            nc.scalar.dma_start(out=lnb_bc, in_=lnb.to_broadcast((P, D)))
            magic_sb = small.tile([P, 1], I32)
            nc.vector.memset(magic_sb, MAGIC)
            for c in range(CCH):
                nc.gpsimd.dma_start(out=woT_sb[:, c, :],
                                    in_=woT[c * P:(c + 1) * P, :])
            def_dmas = []

            # ================= x tiles (ring, all on sync) =========
            # tile key: (tensor_idx, c, half); half = 1024-token col block
            xt = {}

            def emit_x_dma(ti, x_d, c, half):
                t = xp.tile([P, 1024], BF16, name=f"x{ti}_{c}_{half}",
                            tag="xT", bufs=18)
                xt[(ti, c, half)] = t
                nc.sync.dma_start(
                    out=t, in_=x_d[c * P:(c + 1) * P,
                                   half * 1024:(half + 1) * 1024])

            # batch-0: k first (attention needs all of k), then q, then v;
            # batch-1 after, paced by the ring
            for ti, x_d in ((1, xkT), (0, xqT), (2, xvT)):
                for half in (0, 1):
                    for c in range(CCH):
                        emit_x_dma(ti, x_d, c, half)
            for ti, x_d in ((1, xkT), (0, xqT), (2, xvT)):
                for half in (2, 3):
                    for c in range(CCH):
                        emit_x_dma(ti, x_d, c, half)

            # ================= persistent activations =============
            qT_sb = persist.tile([P, NTOK], BF16)
            kT_sb = persist.tile([P, NTOK], BF16)
            v_sb = persist.tile([P, NJ * B, HPC * (DH + 1)], BF16)
            nc.vector.memset(
                v_sb.rearrange("p t (h u) -> p (t h) u", h=HPC)[:, :, DH:DH + 1],
                1.0)

            # ================= unit emitters =======================
            def qk_proj_unit(ti, b, n):
                """Project 512 tokens (chunk n of batch b) for q or k."""
                w_sb, b_sb, o_sb = ((wq_sb, bq_sb, qT_sb) if ti == 0
                                    else (wk_sb, bk_sb, kT_sb))
                tok0 = b * S + n * 512
                half = tok0 // 1024
                off = tok0 % 1024
                ps = aux_pool.tile([P, 512], F32, tag="aux", bufs=2)
                for c in range(CCH):
                    nc.tensor.matmul(ps, w_sb[:, c, :],
                                     xt[(ti, c, half)][:, off:off + 512],
                                     start=(c == 0), stop=(c == CCH - 1))
                nc.vector.tensor_scalar_add(
                    o_sb[:, tok0:tok0 + 512], ps, b_sb)

            def v_proj_unit(b, it):
                """Project one 128-token tile of batch b into v_sb."""
                tok0 = b * S + it * P
                half = tok0 // 1024
                off = tok0 % 1024
                vt = b * NJ + it
                ps = aux_pool.tile([P, 512], F32, tag="aux", bufs=2)
                for c in range(CCH):
                    nc.tensor.matmul(ps[:, 0:DL],
                                     xt[(2, c, half)][:, off:off + P],
                                     wv_sb[:, c, :],
                                     start=(c == 0), stop=(c == CCH - 1))
                for h in range(HPC):
                    nc.vector.tensor_add(
                        v_sb[:, vt, h * (DH + 1):h * (DH + 1) + DH],
                        ps[:, h * DH:(h + 1) * DH],
                        bv_bc[:, h * DH:(h + 1) * DH])

            # e tiles: ring of 4 buffers shared by both heads
            e_tiles = {}

            def alloc_e(b, i):
                for h in range(HPC):
                    e_tiles[(b, i, h)] = ep.tile(
                        [P, NG, 1024], BF16, name=f"e_{b}_{i}_{h}",
                        tag="e", bufs=4)

            def emit_score_group(b, i, g):
                """Scores + exp for group g (2 j-chunks x 2 heads packed)."""
                tok0 = b * S
                ic0 = tok0 + i * 512
                pss = [sc_pool.tile([P, 1024], F32, name=f"sc_{b}_{i}_{g}_{h}",
                                    tag="sc", bufs=3)
                       for h in range(HPC)]
                mm0 = {}
                for jj in range(2):
                    j = g * 2 + jj
                    jc0 = tok0 + j * P
                    for h in range(HPC):
                        mm = nc.tensor.matmul(
                            pss[h][:, jj * 512:(jj + 1) * 512],
                            kT_sb[h * DH:(h + 1) * DH, jc0:jc0 + P],
                            qT_sb[h * DH:(h + 1) * DH, ic0:ic0 + 512])
                        if jj == 0:
                            mm0[h] = mm
                for h in range(HPC):
                    act = nc.scalar.activation(
                        e_tiles[(b, i, h)][:, g, :], pss[h],
                        mybir.ActivationFunctionType.Exp)
                    # PE completes matmuls in pc order: the exp only needs a
                    # semaphore from the LAST producer matmul. Dropping the
                    # first-producer edge removes its sem-inc, restoring
                    # PE tile-pair co-execution (2 heads concurrently).
                    try:
                        act.ins.try_remove_dependency(mm0[h].ins.name)
                    except Exception:
                        pass

            cps_ctr = [0]

            def alloc_cps():
                cps_ctr[0] += 1
                return aux_pool.tile([P, 512], F32, name=f"cps{cps_ctr[0]}",
                                     tag="aux", bufs=2)

            def emit_ctx_mms(b, i, h, cps, j0, j1, first=True, last=True):
                # reversed j: the first mm waits the newest exp; remaining
                # waits are dominated and can be elided by the lowering
                for j in range(j1 - 1, j0 - 1, -1):
                    vt = b * NJ + j
                    nc.tensor.matmul(
                        cps[0:DH + 1, :],
                        v_sb[:, vt, h * (DH + 1):(h + 1) * (DH + 1)],
                        e_tiles[(b, i, h)][:, j // 2,
                                           (j % 2) * 512:(j % 2 + 1) * 512],
                        start=(first and j == j1 - 1),
                        stop=(last and j == j0))

            def emit_renorm_a2a(b, i, h, cps):
                """Row-sum reciprocal, renorm, write to a2a_in blocks."""
                ssum = work.tile([1, 512], F32, tag="ssum", bufs=2)
                nc.vector.tensor_copy(ssum, cps[DH:DH + 1, :])
                rcp = work.tile([1, 512], F32, tag="rcp", bufs=2)
                nc.vector.reciprocal_approx_fast(rcp, ssum)
                rcp_bc = work.tile([DH, 512], F32, tag="rbc", bufs=2)
                nc.gpsimd.partition_broadcast(rcp_bc, rcp)
                ctxo = work.tile([DH, 512], BF16, tag="ctxo", bufs=2)
                nc.vector.tensor_mul(ctxo, cps[0:DH, :], rcp_bc)
                for half in range(2):
                    d = 2 * i + half
                    nc.gpsimd.dma_start(
                        out=a2a_in[b][d, h * DH:(h + 1) * DH, :],
                        in_=ctxo[:, half * TPB:(half + 1) * TPB])

            def emit_collective(b):
                nc.gpsimd.collective_compute(
                    "AllToAll", mybir.AluOpType.bypass,
                    replica_groups=[list(range(NCORES))],
                    ins=[a2a_in[b].opt()],
                    outs=[a2a_out[b].opt()])

            # ---- out-projection + LayerNorm for one 128-token tile ----
            resid_ring = {}
            ctxF = {}
            y_tiles = {}

            def op_prep(t):
                rt = work.tile([P, D], BF16, name=f"res{t}", tag="resid",
                               bufs=2)
                resid_ring[t] = rt
                nc.sync.dma_start(out=rt, in_=resid[t * P:(t + 1) * P, :])
                b, tt = divmod(t, NT // B)
                ft = work.tile([P, CCH, P], BF16, name=f"ctxF{t}", tag="ctxF",
                               bufs=2)
                ctxF[t] = ft
                for s in range(NCORES):
                    nc.sync.dma_start(
                        out=ft[:, s, :],
                        in_=a2a_out[b][s, :, tt * P:(tt + 1) * P])
                y_tiles[t] = work.tile([P, D], F32, name=f"y{t}", tag="y",
                                       bufs=2)

            def outproj_mm_unit(t, n):
                ps = aux_pool.tile([P, 512], F32, tag="aux", bufs=2)
                for s in range(NCORES):
                    nc.tensor.matmul(ps, ctxF[t][:, s, :],
                                     woT_sb[:, s, n * 512:(n + 1) * 512],
                                     start=(s == 0), stop=(s == NCORES - 1))
                yt = y_tiles[t]
                nc.vector.tensor_add(yt[:, n * 512:(n + 1) * 512], ps,
                                     resid_ring[t][:, n * 512:(n + 1) * 512])

            def ln_unit(t, tail=False):
                y = y_tiles[t]
                y3 = y.rearrange("p (g d) -> p g d", g=2)
                stats = work.tile([P, 2, 6], F32, tag="stats")
                for g in range(2):
                    nc.vector.bn_stats(out=stats[:, g, :], in_=y3[:, g, :])
                mv = work.tile([P, 2], F32, tag="mv")
                nc.vector.bn_aggr(out=mv, in_=stats)
                # rstd = 1/sqrt(var+eps): bit hack + 3 Newton steps (DVE)
                var = work.tile([P, 1], F32, tag="var")
                nc.vector.tensor_scalar_add(var, mv[:, 1:2], LN_EPS)
                ish = work.tile([P, 1], I32, tag="ish")
                nc.vector.tensor_scalar(
                    out=ish, in0=var.bitcast(I32), scalar1=1, scalar2=None,
                    op0=mybir.AluOpType.arith_shift_right)
                rst = work.tile([P, 1], F32, tag="rst")
                nc.vector.tensor_tensor(out=rst.bitcast(I32), in0=magic_sb,
                                        in1=ish, op=mybir.AluOpType.subtract)
                tmp = work.tile([P, 1], F32, tag="tmp")
                for _ in range(3):
                    nc.vector.tensor_tensor(out=tmp, in0=rst, in1=rst,
                                            op=mybir.AluOpType.mult)
                    nc.vector.tensor_tensor(out=tmp, in0=tmp, in1=var,
                                            op=mybir.AluOpType.mult)
                    nc.vector.tensor_scalar(
                        out=tmp, in0=tmp, scalar1=-0.5, scalar2=1.5,
                        op0=mybir.AluOpType.mult, op1=mybir.AluOpType.add)
                    nc.vector.tensor_tensor(out=rst, in0=rst, in1=tmp,
                                            op=mybir.AluOpType.mult)
                t32 = work.tile([P, D], F32, tag="t32")
                nc.vector.tensor_scalar(
                    out=t32, in0=y, scalar1=mv[:, 0:1], scalar2=rst,
                    op0=mybir.AluOpType.subtract, op1=mybir.AluOpType.mult)
                of = work.tile([P, D], F32, tag="of", bufs=2)
                eng = nc.gpsimd if tail else nc.vector
                eng.tensor_mul(of, t32, lnw_bc)
                eng.tensor_add(of, of, lnb_bc)
                nc.sync.dma_start(out=out[t * P:(t + 1) * P, :], in_=of)

            # ================= phase script ========================
            # k fully + q chunks 0,1 before attention; q chunks 2,3 as
            # the first fillers (scores i2/i3 read them -> pop in i0)
            for n in range(4):
                qk_proj_unit(1, 0, n)
            for n in range(2):
                qk_proj_unit(0, 0, n)

            # filler units run on PE under the exp-bound attention stream.
            # CONSTRAINT: a unit producing data read by later-emitted PE ops
            # must be emitted before them (in-order PE would deadlock).
            fillers_b0 = []
            for n in range(2, 4):
                fillers_b0.append(lambda n=n: qk_proj_unit(0, 0, n))
            for it in range(NJ):
                fillers_b0.append(lambda it=it: v_proj_unit(0, it))
            for n in range(4):
                fillers_b0.append(lambda n=n: qk_proj_unit(1, 1, n))
            for n in range(4):
                fillers_b0.append(lambda n=n: qk_proj_unit(0, 1, n))
            fillers_b1 = []
            for it in range(NJ):
                fillers_b1.append(lambda it=it: v_proj_unit(1, it))
            for t in range(NT // B):
                def op_unit(t=t):
                    op_prep(t)
                    outproj_mm_unit(t, 0)
                    outproj_mm_unit(t, 1)
                    ln_unit(t)
                fillers_b1.append(op_unit)

            pend = {"ctx": None}

            def attention_batch(b, fill, pop_plan):
                # pop_plan[i][g] = filler units to pop at group g of chunk i
                for i in range(NI):
                    alloc_e(b, i)
                    last = (i == NI - 1)
                    if not last:
                        cps = {}
                        for g in range(NG):
                            emit_score_group(b, i, g)
                            pc = pend["ctx"]
                            if pc is not None:
                                if g == 1:
                                    cps[0] = alloc_cps()
                                    emit_ctx_mms(pc[0], pc[1], 0, cps[0],
                                                 0, NJ)
                                    emit_renorm_a2a(pc[0], pc[1], 0, cps[0])
                                elif g == 4:
                                    cps[1] = alloc_cps()
                                    emit_ctx_mms(pc[0], pc[1], 1, cps[1],
                                                 0, NJ)
                                    emit_renorm_a2a(pc[0], pc[1], 1, cps[1])
                            if b == 0 and i >= 1 and g >= 4 and def_dmas:
                                for _ in range(min(2, len(def_dmas))):
                                    def_dmas.pop(0)()
                            k = pop_plan[i].get(g, 0)
                            for _ in range(min(k, len(fill))):
                                fill.pop(0)()
                        pend["ctx"] = (b, i)
                    else:
                        # final i-chunk: interleave pending + own ctx into
                        # the exp stream so the collective launches with
                        # minimal tail after the last exp
                        pc = pend["ctx"]
                        cps = {}
                        pcps = {}
                        for g in range(NG):
                            emit_score_group(b, i, g)
                            if g < 4 and pc is not None:
                                h, jh = g // 2, (g % 2) * 8
                                if jh == 0:
                                    pcps[h] = alloc_cps()
                                emit_ctx_mms(pc[0], pc[1], h, pcps[h],
                                             jh, jh + 8,
                                             first=(jh == 0), last=(jh == 8))
                                if jh == 8:
                                    emit_renorm_a2a(pc[0], pc[1], h, pcps[h])
                            elif g >= 4:
                                j0 = (g - 4) * 4
                                for h in range(HPC):
                                    if j0 == 0:
                                        cps[h] = alloc_cps()
                                    emit_ctx_mms(b, i, h, cps[h], j0, j0 + 4,
                                                 first=(j0 == 0),
                                                 last=(g == NG - 1))
                        for h in range(HPC):
                            emit_renorm_a2a(b, i, h, cps[h])
                        pend["ctx"] = None
                        emit_collective(b)

            # all vproj-b0 during i0 (must emit before ctx(b0,i0) at i1-g1,
            # paced to xv arrival); qproj-b0 n2/n3 + kproj-b1 in i1;
            # qproj-b1 n0 in i2; rest of b1's projections go to b1-i0
            attention_batch(0, fillers_b0, [
                {2: 1, 3: 1, 4: 4, 5: 4, 6: 4, 7: 4},
                {4: 1, 5: 1, 6: 1, 7: 1},
                {2: 1, 3: 1, 4: 1, 5: 1},
                {}])
            attention_batch(1, fillers_b1, [
                {2: 2, 3: 2, 4: 3, 5: 3, 6: 3, 7: 3},
                {4: 1},
                {2: 1},
                {}])
            for f in fillers_b0 + fillers_b1:
                f()

            # tail: out-proj + LN for batch-1 token tiles
            for t in range(NT // B, NT):
                op_prep(t)
                outproj_mm_unit(t, 0)
                outproj_mm_unit(t, 1)
                ln_unit(t, tail=True)

            if debug_dumps:
                for nm, t in (("dbg_qT", qT_sb), ("dbg_kT", kT_sb),
                              ("dbg_v", v_sb)):
                    dout = nc.dram_tensor(nm, list(t.shape), BF16,
                                          kind="ExternalOutput").ap()
                    nc.sync.dma_start(out=dout, in_=t)

    nc.compile()
    return nc


_NC_CACHE = {}


def _get_nc(S=2048, B=2, D=1024):
    key = (S, B, D)
    if key not in _NC_CACHE:
        _NC_CACHE[key] = build_bert_kernel(S, B, D)
    return _NC_CACHE[key]


def make_in_maps(query_tensor, key_tensor, value_tensor, Wq, bq, Wk, bk,
                 Wv, bv, Wo, bo, ln_w, ln_b):
    S, B, D = query_tensor.shape
    NTOK = S * B
    TPB = S // NCORES
    DL = (H // NCORES) * DH

    def bm(x):  # (S, B, D) -> batch-major (B*S, D) float32
        return np.ascontiguousarray(
            np.asarray(x, np.float32).transpose(1, 0, 2).reshape(NTOK, D))

    def bmT(x):  # feature-major bf16 (D, B*S)
        return np.ascontiguousarray(bm(x).T.astype(BF16_NP))

    xq = bm(query_tensor)
    xqT, xkT, xvT = bmT(query_tensor), bmT(key_tensor), bmT(value_tensor)
    woT = np.ascontiguousarray(
        np.asarray(Wo, np.float32).T.astype(BF16_NP))
    f32 = lambda a: np.ascontiguousarray(np.asarray(a, np.float32))
    bf16T = lambda a: np.ascontiguousarray(
        np.asarray(a, np.float32).T.astype(BF16_NP))
    bo_row = np.asarray(bo, np.float32).reshape(1, D)
    in_maps = []
    for c in range(NCORES):
        sl = slice(c * DL, (c + 1) * DL)
        rows = np.concatenate([
            xq[c * TPB:(c + 1) * TPB],
            xq[S + c * TPB:S + (c + 1) * TPB]], axis=0) + bo_row
        in_maps.append({
            "xqT": xqT, "xkT": xkT, "xvT": xvT,
            "wqT": bf16T(Wq[sl]), "wkT": bf16T(Wk[sl]),
            "wvT": bf16T(Wv[sl]), "woT": woT,
            "bq": f32(bq[sl]).reshape(DL, 1),
            "bk": f32(bk[sl]).reshape(DL, 1),
            "bv": f32(bv[sl]).reshape(1, DL),
            "lnw": f32(ln_w).reshape(1, D),
            "lnb": f32(ln_b).reshape(1, D),
            "resid": np.ascontiguousarray(rows.astype(BF16_NP)),
        })
    return in_maps


def assemble_output(results, S, B, D):
    TPB = S // NCORES
    full = np.empty((B * S, D), np.float32)
    for c, r in enumerate(results):
        o = r["out"]
        full[c * TPB:(c + 1) * TPB] = o[0:TPB]
        full[S + c * TPB:S + (c + 1) * TPB] = o[TPB:2 * TPB]
    return np.ascontiguousarray(
        full.reshape(B, S, D).transpose(1, 0, 2))


def kernel(**inputs):
    S, B, D = inputs["query_tensor"].shape
    nc = _get_nc(S, B, D)
    in_maps = make_in_maps(**inputs)
    res = run_bass_kernel_spmd(nc, in_maps, list(range(NCORES)))
    return assemble_output(res.results, S, B, D)


# revision 45
# speedup vs baseline: 1.0990x; 1.0990x over previous
"""BertAttention Trainium2 kernel — 8-core SPMD, v2.

Sharding: each core owns 2 heads (128 of 1024 feature dims) for QKV +
attention, and a strided 512-token slice (256 tokens from each batch)
for the output projection + LayerNorm.

Key optimizations over v1:
  - score matmuls for the 2 local heads run CONCURRENTLY in the PE array
    (row-tiled at partitions 0-63 / 64-127), 2x score throughput
  - softmax exp (the ACT-engine bottleneck) is pipelined stall-free via a
    3-deep PSUM ring; ctx matmuls lag one i-chunk behind the score/exp
    stream; projections and out-proj run as PE filler under the exp stream
  - ctx keeps a ones-column in V so softmax row-sums are free
  - AllToAll (not AllGather) exchanges exactly the needed ctx slices;
    one per batch, the first fully overlapped with batch-1 attention
  - LayerNorm rsqrt via Newton iterations on DVE (no ACT table switch
    mid-exp-stream); out-proj + LN of batch-0 tokens overlap batch-1
  - last i-chunk of each batch interleaves ctx into the exp stream so the
    collective can start right after the last exp
"""

import os
import sys

for _p in ("/opt/trn_rl_repo", "/root/.axon_site/_ro/trn_rl_repo"):
    if os.path.isdir(_p) and _p not in sys.path:
        sys.path.append(_p)

import ml_dtypes
import numpy as np

try:
    import antenv.axon_hooks  # noqa: F401
except Exception:
    import types as _types
    try:
        import antenv as _antenv
        _m = _types.ModuleType("antenv.axon_hooks")
        _m._hook = None
        _m.set_axon_ntff_profile_hook = lambda h, _m=_m: setattr(_m, "_hook", h)
        _m.get_axon_ntff_profile_hook = lambda _m=_m: _m._hook
        sys.modules["antenv.axon_hooks"] = _m
        _antenv.axon_hooks = _m
    except Exception:
        pass

import concourse.bass as bass
import concourse.tile as tile
from concourse import bacc, mybir
from concourse.bass_utils import run_bass_kernel_spmd

F32 = mybir.dt.float32
I32 = mybir.dt.int32
BF16 = mybir.dt.bfloat16
BF16_NP = ml_dtypes.bfloat16

NCORES = 8
H = 16   # heads total
DH = 64  # head dim
LN_EPS = 1e-12
MAGIC = 0x5F3759DF


def build_bert_kernel(S=2048, B=2, D=1024, debug_dumps=False):
    P = 128
    NTOK = S * B             # batch-major tokens
    TPB = S // NCORES        # tokens per core per batch (256)
    TPC = B * TPB            # tokens per core total (512)
    CCH = D // P             # contraction chunks (8)
    HPC = H // NCORES        # heads per core (2)
    DL = HPC * DH            # local feature dims (128)
    NI = S // 512            # i-chunks per batch (4)
    NG = 8                   # exp groups per i-chunk (2 j-chunks each)
    NJ = S // P              # key chunks per batch (16)
    NT = TPC // P            # out-proj token tiles per core (4)

    nc = bacc.Bacc("TRN2", target_bir_lowering=False, debug=False,
                   num_devices=NCORES)

    def din(name, shape, dt=F32):
        return nc.dram_tensor(name, list(shape), dt, kind="ExternalInput").ap()

    xqT = din("xqT", (D, NTOK), BF16)
    xkT = din("xkT", (D, NTOK), BF16)
    xvT = din("xvT", (D, NTOK), BF16)
    wqT = din("wqT", (D, DL), BF16)
    wkT = din("wkT", (D, DL), BF16)
    wvT = din("wvT", (D, DL), BF16)
    woT = din("woT", (D, D), BF16)
    bq = din("bq", (DL, 1))
    bk = din("bk", (DL, 1))
    bv = din("bv", (1, DL))
    lnw = din("lnw", (1, D))
    lnb = din("lnb", (1, D))
    resid = din("resid", (TPC, D), BF16)  # xq rows + bo pre-added on host
    out = nc.dram_tensor("out", [TPC, D], F32, kind="ExternalOutput").ap()

    # AllToAll buffers: [dest core, local dims (2 heads), dest's tokens]
    a2a_in = [nc.dram_tensor(f"a2a_in{b}", [NCORES, DL, TPB], BF16).ap()
              for b in range(B)]
    a2a_out = [nc.dram_tensor(f"a2a_out{b}", [NCORES, DL, TPB], BF16).ap()
               for b in range(B)]

    with tile.TileContext(nc) as tc:
        with (
            tc.tile_pool(name="persist", bufs=1) as persist,
            tc.tile_pool(name="small", bufs=1) as small,
            tc.tile_pool(name="xp", bufs=1) as xp,
            tc.tile_pool(name="ep", bufs=1) as ep,
            tc.tile_pool(name="sc_pool", bufs=1, space="PSUM") as sc_pool,
            tc.tile_pool(name="aux_pool", bufs=1, space="PSUM") as aux_pool,
            tc.tile_pool(name="work", bufs=1) as work,
        ):
            # ====== weights + consts (scalar/gpsimd; sync is for x) ======
            wq_sb = persist.tile([P, CCH, DL], BF16)
            wk_sb = persist.tile([P, CCH, DL], BF16)
            wv_sb = persist.tile([P, CCH, DL], BF16)
            for c in range(CCH):
                nc.scalar.dma_start(out=wk_sb[:, c, :],
                                    in_=wkT[c * P:(c + 1) * P, :])
            for c in range(CCH):
                nc.scalar.dma_start(out=wq_sb[:, c, :],
                                    in_=wqT[c * P:(c + 1) * P, :])
            for c in range(CCH):
                nc.gpsimd.dma_start(out=wv_sb[:, c, :],
                                    in_=wvT[c * P:(c + 1) * P, :])
            woT_sb = persist.tile([P, CCH, D], BF16)  # loaded mid-attention

            prime = small.tile([1, 8], F32)
            nc.vector.memset(prime, 0.0)
            nc.scalar.activation(prime, prime,
                                 mybir.ActivationFunctionType.Exp)
            bq_sb = small.tile([DL, 1], F32)
            bk_sb = small.tile([DL, 1], F32)
            nc.scalar.dma_start(out=bq_sb, in_=bq)
            nc.scalar.dma_start(out=bk_sb, in_=bk)
            bv_bc = small.tile([P, DL], F32)
            nc.scalar.dma_start(out=bv_bc, in_=bv.to_broadcast((P, DL)))
            lnw_bc = small.tile([P, D], F32)
            nc.scalar.dma_start(out=lnw_bc, in_=lnw.to_broadcast((P, D)))
            lnb_bc = small.tile([P, D], F32)
            nc.scalar.dma_start(out=lnb_bc, in_=lnb.to_broadcast((P, D)))
            magic_sb = small.tile([P, 1], I32)
            nc.vector.memset(magic_sb, MAGIC)
            for c in range(CCH):
                nc.gpsimd.dma_start(out=woT_sb[:, c, :],
                                    in_=woT[c * P:(c + 1) * P, :])
            def_dmas = []

            # ================= x tiles (ring, all on sync) =========
            # tile key: (tensor_idx, c, half); half = 1024-token col block
            xt = {}

            def emit_x_dma(ti, x_d, c, half):
                t = xp.tile([P, 1024], BF16, name=f"x{ti}_{c}_{half}",
                            tag="xT", bufs=18)
                xt[(ti, c, half)] = t
                nc.sync.dma_start(
                    out=t, in_=x_d[c * P:(c + 1) * P,
                                   half * 1024:(half + 1) * 1024])

            # batch-0: k first (attention needs all of k), then q, then v;
            # batch-1 after, paced by the ring
            for ti, x_d in ((1, xkT), (0, xqT), (2, xvT)):
                for half in (0, 1):
                    for c in range(CCH):
                        emit_x_dma(ti, x_d, c, half)
            for ti, x_d in ((1, xkT), (0, xqT), (2, xvT)):
                for half in (2, 3):
                    for c in range(CCH):
                        emit_x_dma(ti, x_d, c, half)

            # ================= persistent activations =============
            qT_sb = persist.tile([P, NTOK], BF16)
            kT_sb = persist.tile([P, NTOK], BF16)
            v_sb = persist.tile([P, NJ * B, HPC * (DH + 1)], BF16)
            nc.vector.memset(
                v_sb.rearrange("p t (h u) -> p (t h) u", h=HPC)[:, :, DH:DH + 1],
                1.0)

            # ================= unit emitters =======================
            def qk_proj_unit(ti, b, n):
                """Project 512 tokens (chunk n of batch b) for q or k."""
                w_sb, b_sb, o_sb = ((wq_sb, bq_sb, qT_sb) if ti == 0
                                    else (wk_sb, bk_sb, kT_sb))
                tok0 = b * S + n * 512
                half = tok0 // 1024
                off = tok0 % 1024
                ps = aux_pool.tile([P, 512], F32, tag="aux", bufs=2)
                for c in range(CCH):
                    nc.tensor.matmul(ps, w_sb[:, c, :],
                                     xt[(ti, c, half)][:, off:off + 512],
                                     start=(c == 0), stop=(c == CCH - 1))
                nc.vector.tensor_scalar_add(
                    o_sb[:, tok0:tok0 + 512], ps, b_sb)

            def v_proj_unit(b, it):
                """Project one 128-token tile of batch b into v_sb."""
                tok0 = b * S + it * P
                half = tok0 // 1024
                off = tok0 % 1024
                vt = b * NJ + it
                ps = aux_pool.tile([P, 512], F32, tag="aux", bufs=2)
                for c in range(CCH):
                    nc.tensor.matmul(ps[:, 0:DL],
                                     xt[(2, c, half)][:, off:off + P],
                                     wv_sb[:, c, :],
                                     start=(c == 0), stop=(c == CCH - 1))
                for h in range(HPC):
                    nc.vector.tensor_add(
                        v_sb[:, vt, h * (DH + 1):h * (DH + 1) + DH],
                        ps[:, h * DH:(h + 1) * DH],
                        bv_bc[:, h * DH:(h + 1) * DH])

            # e tiles: ring of 4 buffers shared by both heads
            e_tiles = {}

            def alloc_e(b, i):
                for h in range(HPC):
                    e_tiles[(b, i, h)] = ep.tile(
                        [P, NG, 1024], BF16, name=f"e_{b}_{i}_{h}",
                        tag="e", bufs=4)

            def emit_score_group(b, i, g):
                """Scores + exp for group g (2 j-chunks x 2 heads packed)."""
                tok0 = b * S
                ic0 = tok0 + i * 512
                pss = [sc_pool.tile([P, 1024], F32, name=f"sc_{b}_{i}_{g}_{h}",
                                    tag="sc", bufs=3)
                       for h in range(HPC)]
                mm0 = {}
                for jj in range(2):
                    j = g * 2 + jj
                    jc0 = tok0 + j * P
                    for h in range(HPC):
                        mm = nc.tensor.matmul(
                            pss[h][:, jj * 512:(jj + 1) * 512],
                            kT_sb[h * DH:(h + 1) * DH, jc0:jc0 + P],
                            qT_sb[h * DH:(h + 1) * DH, ic0:ic0 + 512])
                        if jj == 0:
                            mm0[h] = mm
                for h in range(HPC):
                    act = nc.scalar.activation(
                        e_tiles[(b, i, h)][:, g, :], pss[h],
                        mybir.ActivationFunctionType.Exp)
                    # PE completes matmuls in pc order: the exp only needs a
                    # semaphore from the LAST producer matmul. Dropping the
                    # first-producer edge removes its sem-inc, restoring
                    # PE tile-pair co-execution (2 heads concurrently).
                    try:
                        act.ins.try_remove_dependency(mm0[h].ins.name)
                    except Exception:
                        pass

            cps_ctr = [0]

            def alloc_cps():
                cps_ctr[0] += 1
                return aux_pool.tile([P, 512], F32, name=f"cps{cps_ctr[0]}",
                                     tag="aux", bufs=2)

            def emit_ctx_mms(b, i, h, cps, j0, j1, first=True, last=True):
                # reversed j: the first mm waits the newest exp; remaining
                # waits are dominated and can be elided by the lowering
                for j in range(j1 - 1, j0 - 1, -1):
                    vt = b * NJ + j
                    nc.tensor.matmul(
                        cps[0:DH + 1, :],
                        v_sb[:, vt, h * (DH + 1):(h + 1) * (DH + 1)],
                        e_tiles[(b, i, h)][:, j // 2,
                                           (j % 2) * 512:(j % 2 + 1) * 512],
                        start=(first and j == j1 - 1),
                        stop=(last and j == j0))

            def emit_renorm_a2a(b, i, h, cps):
                """Row-sum reciprocal, renorm, write to a2a_in blocks."""
                ssum = work.tile([1, 512], F32, tag="ssum", bufs=2)
                nc.vector.tensor_copy(ssum, cps[DH:DH + 1, :])
                rcp = work.tile([1, 512], F32, tag="rcp", bufs=2)
                nc.vector.reciprocal_approx_fast(rcp, ssum)
                rcp_bc = work.tile([DH, 512], F32, tag="rbc", bufs=2)
                nc.gpsimd.partition_broadcast(rcp_bc, rcp)
                ctxo = work.tile([DH, 512], BF16, tag="ctxo", bufs=2)
                nc.vector.tensor_mul(ctxo, cps[0:DH, :], rcp_bc)
                for half in range(2):
                    d = 2 * i + half
                    nc.gpsimd.dma_start(
                        out=a2a_in[b][d, h * DH:(h + 1) * DH, :],
                        in_=ctxo[:, half * TPB:(half + 1) * TPB])

            def emit_collective(b):
                nc.gpsimd.collective_compute(
                    "AllToAll", mybir.AluOpType.bypass,
                    replica_groups=[list(range(NCORES))],
                    ins=[a2a_in[b].opt()],
                    outs=[a2a_out[b].opt()])

            # ---- out-projection + LayerNorm for one 128-token tile ----
            resid_ring = {}
            ctxF = {}
            y_tiles = {}

            def op_prep(t):
                rt = work.tile([P, D], BF16, name=f"res{t}", tag="resid",
                               bufs=2)
                resid_ring[t] = rt
                nc.sync.dma_start(out=rt, in_=resid[t * P:(t + 1) * P, :])
                b, tt = divmod(t, NT // B)
                ft = work.tile([P, CCH, P], BF16, name=f"ctxF{t}", tag="ctxF",
                               bufs=2)
                ctxF[t] = ft
                for s in range(NCORES):
                    nc.sync.dma_start(
                        out=ft[:, s, :],
                        in_=a2a_out[b][s, :, tt * P:(tt + 1) * P])
                y_tiles[t] = work.tile([P, D], F32, name=f"y{t}", tag="y",
                                       bufs=2)

            def outproj_mm_unit(t, n):
                ps = aux_pool.tile([P, 512], F32, tag="aux", bufs=2)
                for s in range(NCORES):
                    nc.tensor.matmul(ps, ctxF[t][:, s, :],
                                     woT_sb[:, s, n * 512:(n + 1) * 512],
                                     start=(s == 0), stop=(s == NCORES - 1))
                yt = y_tiles[t]
                nc.vector.tensor_add(yt[:, n * 512:(n + 1) * 512], ps,
                                     resid_ring[t][:, n * 512:(n + 1) * 512])

            def ln_unit(t, tail=False):
                y = y_tiles[t]
                y3 = y.rearrange("p (g d) -> p g d", g=2)
                stats = work.tile([P, 2, 6], F32, tag="stats")
                for g in range(2):
                    nc.vector.bn_stats(out=stats[:, g, :], in_=y3[:, g, :])
                mv = work.tile([P, 2], F32, tag="mv")
                nc.vector.bn_aggr(out=mv, in_=stats)
                # rstd = 1/sqrt(var+eps): bit hack + 3 Newton steps (DVE)
                var = work.tile([P, 1], F32, tag="var")
                nc.vector.tensor_scalar_add(var, mv[:, 1:2], LN_EPS)
                ish = work.tile([P, 1], I32, tag="ish")
                nc.vector.tensor_scalar(
                    out=ish, in0=var.bitcast(I32), scalar1=1, scalar2=None,
                    op0=mybir.AluOpType.arith_shift_right)
                rst = work.tile([P, 1], F32, tag="rst")
                nc.vector.tensor_tensor(out=rst.bitcast(I32), in0=magic_sb,
                                        in1=ish, op=mybir.AluOpType.subtract)
                tmp = work.tile([P, 1], F32, tag="tmp")
                for _ in range(3):
                    nc.vector.tensor_tensor(out=tmp, in0=rst, in1=rst,
                                            op=mybir.AluOpType.mult)
                    nc.vector.tensor_tensor(out=tmp, in0=tmp, in1=var,
                                            op=mybir.AluOpType.mult)
                    nc.vector.tensor_scalar(
                        out=tmp, in0=tmp, scalar1=-0.5, scalar2=1.5,
                        op0=mybir.AluOpType.mult, op1=mybir.AluOpType.add)
                    nc.vector.tensor_tensor(out=rst, in0=rst, in1=tmp,
                                            op=mybir.AluOpType.mult)
                t32 = work.tile([P, D], F32, tag="t32")
                nc.vector.tensor_scalar(
                    out=t32, in0=y, scalar1=mv[:, 0:1], scalar2=rst,
                    op0=mybir.AluOpType.subtract, op1=mybir.AluOpType.mult)
                of = work.tile([P, D], F32, tag="of", bufs=2)
                eng = nc.gpsimd if tail else nc.vector
                eng.tensor_mul(of, t32, lnw_bc)
                eng.tensor_add(of, of, lnb_bc)
                nc.sync.dma_start(out=out[t * P:(t + 1) * P, :], in_=of)

            # ================= phase script ========================
            # k fully + q chunks 0,1 before attention; q chunks 2,3 as
            # the first fillers (scores i2/i3 read them -> pop in i0)
            for n in range(4):
                qk_proj_unit(1, 0, n)
            for n in range(2):
                qk_proj_unit(0, 0, n)

            # filler units run on PE under the exp-bound attention stream.
            # CONSTRAINT: a unit producing data read by later-emitted PE ops
            # must be emitted before them (in-order PE would deadlock).
            fillers_b0 = []
            for n in range(2, 4):
                fillers_b0.append(lambda n=n: qk_proj_unit(0, 0, n))
            for it in range(NJ):
                fillers_b0.append(lambda it=it: v_proj_unit(0, it))
            for n in range(4):
                fillers_b0.append(lambda n=n: qk_proj_unit(1, 1, n))
            for n in range(4):
                fillers_b0.append(lambda n=n: qk_proj_unit(0, 1, n))
            fillers_b1 = []
            for it in range(NJ):
                fillers_b1.append(lambda it=it: v_proj_unit(1, it))
            for t in range(NT // B):
                def op_unit(t=t):
                    op_prep(t)
                    outproj_mm_unit(t, 0)
                    outproj_mm_unit(t, 1)
                    ln_unit(t)
                fillers_b1.append(op_unit)

            pend = {"ctx": None}

            def attention_batch(b, fill, pop_plan):
                # pop_plan[i][g] = filler units to pop at group g of chunk i
                for i in range(NI):
                    alloc_e(b, i)
                    last = (i == NI - 1)
                    if not last:
                        cps = {}
                        for g in range(NG):
                            emit_score_group(b, i, g)
                            pc = pend["ctx"]
                            if pc is not None:
                                if g == 1:
                                    cps[0] = alloc_cps()
                                    emit_ctx_mms(pc[0], pc[1], 0, cps[0],
                                                 0, NJ)
                                    emit_renorm_a2a(pc[0], pc[1], 0, cps[0])
                                elif g == 4:
                                    cps[1] = alloc_cps()
                                    emit_ctx_mms(pc[0], pc[1], 1, cps[1],
                                                 0, NJ)
                                    emit_renorm_a2a(pc[0], pc[1], 1, cps[1])
                            if b == 0 and i >= 1 and g >= 4 and def_dmas:
                                for _ in range(min(2, len(def_dmas))):
                                    def_dmas.pop(0)()
                            k = pop_plan[i].get(g, 0)
                            for _ in range(min(k, len(fill))):
                                fill.pop(0)()
                        pend["ctx"] = (b, i)
                    else:
                        # final i-chunk: interleave pending + own ctx into
                        # the exp stream so the collective launches with
                        # minimal tail after the last exp
                        pc = pend["ctx"]
                        cps = {}
                        pcps = {}
                        for g in range(NG):
                            emit_score_group(b, i, g)
                            if g < 4 and pc is not None:
                                h, jh = g // 2, (g % 2) * 8
                                if jh == 0:
                                    pcps[h] = alloc_cps()
                                emit_ctx_mms(pc[0], pc[1], h, pcps[h],
                                             jh, jh + 8,
                                             first=(jh == 0), last=(jh == 8))
                                if jh == 8:
                                    emit_renorm_a2a(pc[0], pc[1], h, pcps[h])
                            elif g >= 4:
                                j0 = (g - 4) * 4
                                for h in range(HPC):
                                    if j0 == 0:
                                        cps[h] = alloc_cps()
                                    emit_ctx_mms(b, i, h, cps[h], j0, j0 + 4,
                                                 first=(j0 == 0),
                                                 last=(g == NG - 1))
                        for h in range(HPC):
                            emit_renorm_a2a(b, i, h, cps[h])
                        pend["ctx"] = None
                        emit_collective(b)

            # all vproj-b0 during i0 (must emit before ctx(b0,i0) at i1-g1,
            # paced to xv arrival); qproj-b0 n2/n3 + kproj-b1 in i1;
            # qproj-b1 n0 in i2; rest of b1's projections go to b1-i0
            attention_batch(0, fillers_b0, [
                {2: 1, 3: 1, 4: 4, 5: 4, 6: 4, 7: 4},
                {4: 1, 5: 1, 6: 1, 7: 1},
                {2: 1, 3: 1, 4: 1, 5: 1},
                {}])
            attention_batch(1, fillers_b1, [
                {2: 2, 3: 2, 4: 3, 5: 3, 6: 3, 7: 3},
                {6: 1},
                {6: 1},
                {}])
            for f in fillers_b0 + fillers_b1:
                f()

            # tail: out-proj + LN for batch-1 token tiles
            for t in range(NT // B, NT):
                op_prep(t)
                outproj_mm_unit(t, 0)
                outproj_mm_unit(t, 1)
                ln_unit(t, tail=True)

            if debug_dumps:
                for nm, t in (("dbg_qT", qT_sb), ("dbg_kT", kT_sb),
                              ("dbg_v", v_sb)):
                    dout = nc.dram_tensor(nm, list(t.shape), BF16,
                                          kind="ExternalOutput").ap()
                    nc.sync.dma_start(out=dout, in_=t)

    nc.compile()
    return nc


_NC_CACHE = {}


def _get_nc(S=2048, B=2, D=1024):
    key = (S, B, D)
    if key not in _NC_CACHE:
        _NC_CACHE[key] = build_bert_kernel(S, B, D)
    return _NC_CACHE[key]


def make_in_maps(query_tensor, key_tensor, value_tensor, Wq, bq, Wk, bk,
                 Wv, bv, Wo, bo, ln_w, ln_b):
    S, B, D = query_tensor.shape
    NTOK = S * B
    TPB = S // NCORES
    DL = (H // NCORES) * DH

    def bm(x):  # (S, B, D) -> batch-major (B*S, D) float32
        return np.ascontiguousarray(
            np.asarray(x, np.float32).transpose(1, 0, 2).reshape(NTOK, D))

    def bmT(x):  # feature-major bf16 (D, B*S)
        return np.ascontiguousarray(bm(x).T.astype(BF16_NP))

    xq = bm(query_tensor)
    xqT, xkT, xvT = bmT(query_tensor), bmT(key_tensor), bmT(value_tensor)
    woT = np.ascontiguousarray(
        np.asarray(Wo, np.float32).T.astype(BF16_NP))
    f32 = lambda a: np.ascontiguousarray(np.asarray(a, np.float32))
    bf16T = lambda a: np.ascontiguousarray(
        np.asarray(a, np.float32).T.astype(BF16_NP))
    bo_row = np.asarray(bo, np.float32).reshape(1, D)
    in_maps = []
    for c in range(NCORES):
        sl = slice(c * DL, (c + 1) * DL)
        rows = np.concatenate([
            xq[c * TPB:(c + 1) * TPB],
            xq[S + c * TPB:S + (c + 1) * TPB]], axis=0) + bo_row
        in_maps.append({
            "xqT": xqT, "xkT": xkT, "xvT": xvT,
            "wqT": bf16T(Wq[sl]), "wkT": bf16T(Wk[sl]),
            "wvT": bf16T(Wv[sl]), "woT": woT,
            "bq": f32(bq[sl]).reshape(DL, 1),
            "bk": f32(bk[sl]).reshape(DL, 1),
            "bv": f32(bv[sl]).reshape(1, DL),
            "lnw": f32(ln_w).reshape(1, D),
            "lnb": f32(ln_b).reshape(1, D),
            "resid": np.ascontiguousarray(rows.astype(BF16_NP)),
        })
    return in_maps


def assemble_output(results, S, B, D):
    TPB = S // NCORES
    full = np.empty((B * S, D), np.float32)
    for c, r in enumerate(results):
        o = r["out"]
        full[c * TPB:(c + 1) * TPB] = o[0:TPB]
        full[S + c * TPB:S + (c + 1) * TPB] = o[TPB:2 * TPB]
    return np.ascontiguousarray(
        full.reshape(B, S, D).transpose(1, 0, 2))


def kernel(**inputs):
    S, B, D = inputs["query_tensor"].shape
    nc = _get_nc(S, B, D)
    in_maps = make_in_maps(**inputs)
    res = run_bass_kernel_spmd(nc, in_maps, list(range(NCORES)))
    return assemble_output(res.results, S, B, D)
